# revision 54
# baseline (speedup 1.0000x reference)
"""Trainium2 Bass kernel for the grouped contrastive loss.

Math: for anchors i and positives j restricted to the same
sensitive-attribute group g (size P),
    row(i,j) = S_ij - D * log E_ij
with S_ij = <p_i, p_j>/t and E_ij = sum_d exp(p_i[d] p_j[d] / t)
(the log-softmax max-shift cancels analytically), and
    loss = sum_i -1/(N P_i^2) * sum_{j in g(i)} row(i,j).

row(i,j) is symmetric, so the group's P x P matrix is covered by
chunking each group into <=128-column chunks: the diagonal chunk-square
is computed in full at weight 1 and cross chunk pairs only once (rows of
earlier chunks x cols of later chunk) at weight 2 -- B(B+1)/2 slots per
group instead of B^2. Slot = up to 128 anchor rows x one col chunk
(<=128 cols), rows packed 128-at-a-time from all chunks <= the col
chunk. Slots are sorted by real column count and assigned round-robin,
so each program position uses the max real width at that position
(compile-time, same on all cores) -- partial windows cost only what
they cover. Narrow positions are interleaved mid-stream so their
overhead-bound PE chains hide under wide slots' EXPs.

Per slot, on device (anchors on partitions as 32 packs of 4 anchors x
32 dims):
  - sum_j S_ij = <a_i, sum_j w_j> directly via one N=1 bf16 matmul
    against the host-precomputed window-sum vector (no [128,128] S tile
    at all), accumulated into a [128, ntiles] PSUM column tile.
  - prod via ONE DVE tensor_tensor per slot: scalars stored duplicated
    in pairs (scal2[p,2k]=scal2[p,2k+1]) so all three operands' APs end
    in a packed [1,2] bf16 dim -> DVE 2x mode; stride-0 outer dims do
    the pack/window broadcast. Then one batched ACT Exp ([128, 4096]
    bf16), and per-pack bf16 matmuls against shifted block-diagonal
    ones accumulating each anchor's 32 exp rows into its PSUM row
    (4 chains x 8 packs into one [128,128] PSUM tile via explicit
    tile_position). Slot 0 is sub-chunked 4x to shorten the startup
    ramp.
  - Ln on ACT; DVE row-reduction of log E into a [128, ntiles] column
    tile. The raw SL/SS column tiles are DMA'd out directly; the host
    applies the per-(row,slot) weights and reduces (no device epilogue
    on the tail's critical path).
A manually pre-placed InstLoadActFuncSet of the combined exp+ln table
avoids the per-switch ACT table reloads. Dummy rows/cols are weighted
out (w=0); the exact -D*ln(D)*n_dummy-per-slot correction is added by
the host. The 8 cores run one SPMD program; the host sums the [128]
partials.
"""

import math
import os
import sys

sys.path.insert(0, "/opt/trn_rl_repo")

import numpy as np
import ml_dtypes

import concourse.bacc as bacc
import concourse.bass as bass
import concourse.tile as tile
from concourse import mybir
from concourse.bass_utils import run_bass_kernel_spmd

N_CORES = 8
D = 32
W = 128  # window (col chunk) width
PACKS = 32  # packs of 4 anchors per 128-anchor slot

last_run_info = {}


def _install_ntff_hook():
    # bass_utils' trace path under axon imports antenv.axon_hooks, which is
    # absent in this image; provide the ctypes-based hook it expects.
    import contextlib
    import ctypes
    import types

    if "antenv.axon_hooks" in sys.modules:
        return

    def _make_hook():
        try:
            lib = ctypes.CDLL("/opt/axon/libaxon_pjrt.so")
        except OSError:
            return None
        if not hasattr(lib, "axon_start_nrt_profile"):
            return None
        lib.axon_start_nrt_profile.argtypes = [
            ctypes.POINTER(ctypes.c_int64),
            ctypes.c_size_t,
        ]
        lib.axon_start_nrt_profile.restype = ctypes.c_int64
        lib.axon_stop_nrt_profile.argtypes = [ctypes.c_char_p]
        lib.axon_stop_nrt_profile.restype = ctypes.c_int64

        @contextlib.contextmanager
        def _hook_cm(output_dir, device_ids):
            import jax

            jax.devices()
            if device_ids:
                ids = (ctypes.c_int64 * len(device_ids))(*device_ids)
                rc = lib.axon_start_nrt_profile(ids, len(device_ids))
            else:
                rc = lib.axon_start_nrt_profile(None, 0)
            if rc != 0:
                raise RuntimeError(f"axon_start_nrt_profile rc={rc}")
            try:
                yield
            finally:
                n = lib.axon_stop_nrt_profile(str(output_dir).encode())
                if n < 0:
                    raise RuntimeError(f"axon_stop_nrt_profile rc={n}")

        return _hook_cm

    hook = _make_hook()
    mod = types.ModuleType("antenv.axon_hooks")
    mod.get_axon_ntff_profile_hook = lambda: hook
    mod.set_axon_ntff_profile_hook = lambda h: None
    sys.modules["antenv.axon_hooks"] = mod


def _plan(sa_sorted):
    """Slot plan from the sorted attribute vector.

    Each slot is (rows, weights, c0, L):
      rows: array of <=128 sorted-anchor positions (the slot's anchors)
      weights: per-row pair multiplicity (1 diag chunk, 2 earlier chunk)
      [c0, c0+L): the slot's col window (sorted positions, one chunk)
    or None for a dummy slot. Returns (widths, per_core).
    """
    n = len(sa_sorted)
    bounds = [0]
    for i in range(1, n):
        if sa_sorted[i] != sa_sorted[i - 1]:
            bounds.append(i)
    bounds.append(n)

    slots = []
    for gi in range(len(bounds) - 1):
        g0, g1 = bounds[gi], bounds[gi + 1]
        P = g1 - g0
        B = (P + W - 1) // W
        for w in range(B):
            c0 = g0 + W * w
            L = min(W, g1 - c0)
            r_hi = min(g0 + W * (w + 1), g1)  # rows of chunks 0..w
            rows_all = np.arange(g0, r_hi)
            wts_all = np.where(rows_all < c0, 2.0, 1.0)
            for r0 in range(0, len(rows_all), 128):
                slots.append(
                    (rows_all[r0 : r0 + 128], wts_all[r0 : r0 + 128], c0, L)
                )

    # Assign slots to cores sorted by real width, so each program
    # position can use the max real width at that position (compile-time
    # constant, same for all cores) instead of always-128 -- partial
    # windows then cost what they actually cover.
    slots.sort(key=lambda t: -t[3])
    npos = (len(slots) + N_CORES - 1) // N_CORES
    widths = []
    chunks = []
    for p in range(npos):
        chunk = slots[p * N_CORES : (p + 1) * N_CORES]
        wmax = max(t[3] for t in chunk)
        widths.append(2 * ((wmax + 1) // 2))  # even width for the pair trick
        chunks.append(chunk)

    # interleave: narrow positions go mid-stream so their overhead-bound
    # PE chains hide under wide slots' long EXPs (never first or last)
    order = sorted(range(npos), key=lambda p: -widths[p])
    wide = [p for p in order if widths[p] >= W // 2]
    narrow = [p for p in order if widths[p] < W // 2]
    seq = []
    wi, ni = 0, 0
    for i in range(npos):
        # positions 0,1 and the last stay wide when possible
        take_narrow = ni < len(narrow) and 2 <= i < npos - 1 and (i % 2 == 0)
        if take_narrow or wi >= len(wide):
            seq.append(narrow[ni])
            ni += 1
        else:
            seq.append(wide[wi])
            wi += 1
    widths = [widths[p] for p in seq]
    per_core = [[chunks[p][c] if c < len(chunks[p]) else None for p in seq]
                for c in range(N_CORES)]
    return widths, per_core


def _exp_ln_table_id(nc):
    try:
        from concourse.hw_specs import get_activation_tables

        tabs = get_activation_tables(nc.m.arch)
        Exp = mybir.ActivationFunctionType.Exp
        Ln = mybir.ActivationFunctionType.Ln
        for idx, funcs in enumerate(tabs.values()):
            if Exp in funcs and Ln in funcs:
                return idx
    except Exception:
        pass
    return 6  # natural_log_exp_and_others in this neuronxcc's act_info.json


def _layout(widths):
    """Offsets for the packed 'big' tensor: per-slot [rep4_s | scal2_s]
    blocks, then onesbd."""
    ncols = len(widths)
    offs = []
    o = 0
    for w in widths:
        offs.append(o)
        o += w + 2 * PACKS
    ones_off = o
    big_cols = ones_off + 8 * 32
    biga_cols = ncols * 128 + ncols
    return offs, ones_off, big_cols, biga_cols


def _build_program(widths):
    # Bacc (not raw Bass): its compile() runs generate_event_semaphores,
    # which splits multi-semaphore waits to satisfy the TRN2 one-wait-per-
    # instruction constraint this walrus build enforces.
    nc = bacc.Bacc(
        "TRN2", target_bir_lowering=False, debug=False, num_devices=N_CORES
    )
    f32 = mybir.dt.float32
    bf16 = mybir.dt.bfloat16

    ncols = len(widths)
    offs, ones_off, big_cols, biga_cols = _layout(widths)
    big_d = nc.dram_tensor("big", [128, big_cols], bf16, kind="ExternalInput").ap()
    biga_d = nc.dram_tensor("biga", [32, biga_cols], bf16, kind="ExternalInput").ap()
    out_d = nc.dram_tensor("out", [128, 2 * ncols], f32, kind="ExternalOutput").ap()

    Exp = mybir.ActivationFunctionType.Exp
    Ln = mybir.ActivationFunctionType.Ln

    with tile.TileContext(nc) as tc:
        with (
            tc.tile_pool(name="const", bufs=1) as cpool,
            tc.tile_pool(name="work", bufs=4) as wpool,
            tc.tile_pool(name="psE", bufs=5, space="PSUM") as psE,
            tc.tile_pool(name="psS", bufs=1, space="PSUM") as psS,
        ):
            # preload the combined exp+ln table so Exp/Ln interleaving
            # never reloads activation tables (saves ~1.3us per switch)
            nc.scalar.add_instruction(
                mybir.InstLoadActFuncSet(
                    name=nc.get_next_instruction_name(),
                    ins=[],
                    outs=[],
                    act_func_set_id=_exp_ln_table_id(nc),
                )
            )

            # slot-0/1 slices land first (own DMAs) so compute starts
            # before the bulk transfers; 5 DMA issues across 3 engines
            big = cpool.tile([128, big_cols], bf16, tag="big")
            b1 = offs[1] if ncols > 1 else big_cols
            b2 = offs[2] if ncols > 2 else big_cols
            nc.sync.dma_start(big[:, 0:b1], big_d[:, 0:b1])
            biga = cpool.tile([32, biga_cols], bf16, tag="biga")
            nc.scalar.dma_start(biga[:], biga_d[:])
            if b2 > b1:
                nc.gpsimd.dma_start(big[:, b1:b2], big_d[:, b1:b2])
            mid = b2 + ((big_cols - b2) // 2 // 2) * 2
            if mid > b2:
                nc.gpsimd.dma_start(big[:, b2:mid], big_d[:, b2:mid])
            if big_cols > mid:
                nc.sync.dma_start(big[:, mid:], big_d[:, mid:])

            rep4 = lambda s: big[:, offs[s] : offs[s] + widths[s]]
            scal = lambda s: big[:, offs[s] + widths[s] : offs[s] + widths[s] + 2 * PACKS]
            onesbd = big[:, ones_off : ones_off + 8 * 32]
            lhsa = lambda s: biga[:, s * 128 : (s + 1) * 128]
            wsums = lambda s: biga[:, ncols * 128 + s : ncols * 128 + s + 1]

            SLX = cpool.tile([128, 2 * ncols], f32, tag="SLX")
            SL = SLX[:, 0:ncols]
            SS = psS.tile([128, ncols], f32, tag="SS")

            def mult_exp(rep_ap, scal_ap, k0, k1, prod, expt, width):
                # prod[:, k*width:(k+1)*width] = rep * scal[:,k] for
                # k0<=k<k1, in one 2x-mode DVE op via pair-dup scalars
                nk = k1 - k0
                in0 = (
                    rep_ap.rearrange("p (j2 two) -> p j2 two", two=2)
                    .unsqueeze(1)
                    .broadcast_to([128, nk, width // 2, 2])
                )
                in1 = (
                    scal_ap[:, 2 * k0 : 2 * k1]
                    .rearrange("p (k two) -> p k two", two=2)
                    .unsqueeze(2)
                    .broadcast_to([128, nk, width // 2, 2])
                )
                outp = prod[:, k0 * width : k1 * width].rearrange(
                    "p (k j2 two) -> p k j2 two", k=nk, two=2
                )
                nc.vector.tensor_tensor(outp, in0, in1, op=mybir.AluOpType.mult)
                nc.scalar.activation(
                    expt[:, k0 * width : k1 * width],
                    prod[:, k0 * width : k1 * width],
                    Exp,
                )

            def e_chain(h, E_ps, expt, width):
                for i in range(8):
                    k = 8 * h + i
                    nc.tensor.matmul(
                        E_ps[32 * h : 32 * h + 32, :],
                        lhsT=onesbd[:, 32 * i : 32 * (i + 1)],
                        rhs=expt[:, k * width : (k + 1) * width],
                        start=(i == 0),
                        stop=(i == 7),
                        tile_position=(0, 32 * h),
                    )

            def slot_body(col, rep_ap, scal_ap, lhsa_ap, wsum_ap, width, nsub):
                nc.tensor.matmul(
                    SS[:, col : col + 1],
                    lhsT=lhsa_ap,
                    rhs=wsum_ap,
                    start=True,
                    stop=True,
                )
                prod = wpool.tile([128, PACKS * width], bf16, tag=f"prod{width}")
                expt = wpool.tile([128, PACKS * width], bf16, tag=f"expt{width}")
                E_full = psE.tile([128, W], f32, tag="E")
                E_ps = E_full[:, 0:width]
                if nsub == 4:
                    for h in range(4):
                        mult_exp(rep_ap, scal_ap, 8 * h, 8 * (h + 1), prod, expt, width)
                        e_chain(h, E_ps, expt, width)
                else:
                    mult_exp(rep_ap, scal_ap, 0, PACKS, prod, expt, width)
                    for h in range(4):
                        e_chain(h, E_ps, expt, width)
                # Ln in place over E (PSUM): saves the psL pool + a copy
                nc.scalar.activation(E_ps, E_ps, Ln)
                nc.vector.tensor_reduce(
                    SL[:, col : col + 1],
                    E_ps,
                    axis=mybir.AxisListType.X,
                    op=mybir.AluOpType.add,
                )

            for s in range(ncols):
                slot_body(
                    s,
                    rep4(s),
                    scal(s),
                    lhsa(s),
                    wsums(s),
                    widths[s],
                    4 if (s == 0 or (s == ncols - 1 and widths[s] >= 64)) else 1,
                )

            # ship raw SL and SS columns; host applies the weights
            nc.vector.tensor_copy(SLX[:, ncols : 2 * ncols], SS[:])
            nc.sync.dma_start(out_d[:], SLX[:])

    nc.compile()
    return nc


def kernel(points, sensitive_attribute, t):
    _install_ntff_hook()

    points = np.asarray(points, dtype=np.float32)
    sa = np.asarray(sensitive_attribute).astype(np.int64)
    n, d = points.shape
    assert d == D

    scale = 1.0 / math.sqrt(float(np.asarray(t)))
    order = np.argsort(sa, kind="stable")
    sa_sorted = sa[order]
    ps = (points[order] * np.float32(scale)).astype(np.float32)  # [n, 32] sorted
    ps_bf = ps.astype(ml_dtypes.bfloat16)

    # group size per sorted position (for the 1/P^2 weights)
    _, counts = np.unique(sa_sorted, return_counts=True)
    gsize = np.repeat(counts, counts).astype(np.float64)

    widths, per_core = _plan(sa_sorted)
    ncols = len(widths)

    lnD = math.log(float(D))
    onesbd = np.zeros((128, 8 * 32), ml_dtypes.bfloat16)
    for i in range(8):
        for a in range(4):
            onesbd[32 * a : 32 * (a + 1), 32 * i + 4 * i + a] = 1.0

    in_maps = []
    host_const = 0.0  # sum of per-row dummy-col corrections (exact)

    def pack_slot(slot, width):
        """-> (rep [128,width], sc2 [128,64], ablk16 [32,128], wsum [32],
        wcol_r [R], correction)"""
        rows, wts, c0, L = slot
        R = len(rows)
        rep = np.zeros((128, width), ml_dtypes.bfloat16)
        win = ps_bf[c0 : c0 + L].T  # [32, L]
        rep[:, :L] = np.tile(win, (4, 1))
        wsum = win.astype(np.float32).sum(axis=1).astype(ml_dtypes.bfloat16)
        ablk = np.zeros((32, 128), np.float32)
        ablk[:, :R] = ps[rows].T
        # scal column k = slot rows 4k..4k+3 flattened (a-major, d-minor),
        # stored twice (pair duplication for DVE 2x)
        sc = ablk.T.reshape(PACKS, 128).T.astype(ml_dtypes.bfloat16)
        P = gsize[rows]
        wcol_r = -wts / (n * P * P)
        corr = float(np.sum(wcol_r) * (D * lnD) * (width - L))
        return rep, np.repeat(sc, 2, axis=1), ablk.astype(
            ml_dtypes.bfloat16
        ), wsum, wcol_r, corr

    # big/biga packed layouts (must match _build_program)
    offs, ones_off, big_cols, biga_cols = _layout(widths)
    wmats = []

    for c in range(N_CORES):
        big = np.zeros((128, big_cols), ml_dtypes.bfloat16)
        biga = np.zeros((32, biga_cols), ml_dtypes.bfloat16)
        AB = np.zeros((128, 2 * ncols), np.float32)
        for s, slot in enumerate(per_core[c]):
            if slot is None:
                continue
            ws = widths[s]
            rep, sc2, ablk16, wsum, wcol_r, corr = pack_slot(slot, ws)
            R = len(wcol_r)
            big[:, offs[s] : offs[s] + ws] = rep
            big[:, offs[s] + ws : offs[s] + ws + 2 * PACKS] = sc2
            biga[:, s * 128 : (s + 1) * 128] = ablk16
            biga[:, ncols * 128 + s] = wsum
            AB[:R, s] = wcol_r
            AB[:R, ncols + s] = -float(D) * wcol_r
            host_const += corr
        big[:, ones_off : ones_off + 8 * 32] = onesbd
        in_maps.append({"big": big, "biga": biga})
        wmats.append(AB)

    nc = _build_program(widths)
    trace = bool(int(os.environ.get("KERNEL_TRACE", "0")))
    try:
        res = run_bass_kernel_spmd(nc, in_maps, list(range(N_CORES)), trace=trace)
    except Exception:
        # one retry: shields against a transiently wedged device state
        res = run_bass_kernel_spmd(nc, in_maps, list(range(N_CORES)), trace=trace)
    last_run_info["exec_time_ns"] = res.exec_time_ns
    last_run_info["mean_exec_time_ns"] = res.mean_exec_time_ns
    last_run_info["W"] = W
    last_run_info["ntiles"] = ncols
    last_run_info["widths"] = widths
    last_run_info["instructions"] = (
        res.instructions_and_trace[0] if res.instructions_and_trace else None
    )

    total = float(host_const)
    for c in range(N_CORES):
        out = res.results[c]["out"].astype(np.float64)  # [128, 2*ncols]
        SLh = out[:, 0:ncols]
        SSh = out[:, ncols : 2 * ncols]
        AB = wmats[c].astype(np.float64)
        A = AB[:, 0:ncols]
        B = AB[:, ncols : 2 * ncols]
        total += float((A * SSh + B * SLh).sum())
    return np.float32(total)
